# revision 1
# baseline (speedup 1.0000x reference)
"""Trainium2 Bass kernel for nn_ContactAttention (B=2, L=512, E=128).

Strategy:
  - Front-end (conv1d x2 + BN + 3 transformer encoder layers) replicated on
    all 8 cores (it is tiny); all heavy matmuls in bf16, fp32 psum/stats.
  - The (B, 2E+2, L, L) pairwise tensor is never materialized. Using
    seq_mat[b,c,i,j] = emb[b,c,min(i,j)] (c<128) / emb[b,c-128,max(i,j)],
    contact rows are computed from emb and streamed lc_w rows:
      contact[b,i,j] = sum_c emb[b,c,j]*Wa[c,i,j] + sum_c emb[b,c,i]*Wb[c,i,j]
                       + pos terms
    where Wa/Wb are the host-side min/max channel selections of lc_w.
  - Row-sharded over 8 cores (64 rows each). Per row: a vector elementwise
    product emb (.) Wa_row reduced over channels by a ones-column matmul, and
    the emb[:,i]-term computed directly as a matmul; results accumulate into
    one (128, 512) psum block (output row placed via a shifting-window lhsT).
  - Host: shard/gather + final symmetrization (c + c^T)/2.
"""
import sys

sys.path.insert(0, "/opt/trn_rl_repo")

import contextlib

import numpy as np
import ml_dtypes

import concourse.bass as bass
import concourse.mybir as mybir
import concourse.tile as tile
from concourse import bacc
from concourse.bass_utils import run_bass_kernel_spmd
from concourse.masks import make_identity

# Route every activation function to the one table set that contains them
# all (natural_log_exp_and_others): the stock greedy placement alternates
# exp_and_others <-> natural_log, reloading ACT tables (~2.7us) per switch.
import concourse.hw_specs as _hw_specs

_orig_gat = _hw_specs.get_activation_tables


def _gat_pinned(module_arch):
    t = _orig_gat(module_arch)
    out = {}
    for name, fns in t.items():
        if name == "natural_log_exp_and_others":
            out[name] = fns
        else:
            out[name] = set()
    return out


_hw_specs.get_activation_tables = _gat_pinned
try:
    import concourse.bacc as _bacc_mod
    if hasattr(_bacc_mod, "get_activation_tables"):
        _bacc_mod.get_activation_tables = _gat_pinned
except Exception:
    pass

BF = ml_dtypes.bfloat16
dt = mybir.dt
F32, BF16 = dt.float32, dt.bfloat16
AL = mybir.AluOpType
AF = mybir.ActivationFunctionType

B, L, D, E, H, HD, FF = 2, 512, 127, 128, 2, 64, 2048
NCORES = 8
RPC = L // NCORES  # 64 rows per core
EPS = 1e-5
WCHUNK = 4  # lc rows per DMA chunk

_cached = {}


def _build():
    nc = bacc.Bacc("TRN2", target_bir_lowering=False, debug=False,
                   num_devices=NCORES)

    def din(name, shape, d=F32):
        return nc.dram_tensor(name, shape, d, kind="ExternalInput")

    # ---- replicated inputs ----
    seqT = din("seqT", (4, 2 * L), BF16)
    w1Tt = din("w1Tt", (4, 9 * D), BF16)
    w2Tt = din("w2Tt", (D, 9 * D), BF16)
    bn1g = din("bn1g", (D, 1)); bn1b = din("bn1b", (D, 1))
    bn2g = din("bn2g", (D, 1)); bn2b = din("bn2b", (D, 1))
    rvec_bf = din("rvec_bf", (1, L), BF16)
    rvec_f = din("rvec_f", (1, L))
    iwTq = din("iwTq", (E, 3 * E), BF16)
    iwTk = din("iwTk", (E, 3 * E), BF16)
    iwTv = din("iwTv", (E, 3 * E), BF16)
    qkb = din("qkb", (E, 6))              # col 2l = qb_l, 2l+1 = kb_l
    owTd = din("owT", (64, 3 * 2 * E), BF16)
    obe = din("obe", (E, 3))
    ln1g = din("ln1g", (E, 3)); ln1b = din("ln1b", (E, 3))
    ln2g = din("ln2g", (E, 3)); ln2b = din("ln2b", (E, 3))
    w1Tf = din("w1Tf", (E, 3 * FF), BF16)
    fb1 = din("fb1", (E, 3 * 16))
    w2Tf = din("w2Tf", (E, 3 * 16 * E), BF16)
    fb2 = din("fb2", (E, 3))
    # ---- per-core inputs ----
    wa_d = din("wa", (128, RPC * L), BF16)
    wb_d = din("wb", (128, RPC * L), BF16)
    lcp256 = din("lcp256", (RPC, L))
    lcp257 = din("lcp257", (RPC, L))
    rcol = din("rcol", (128, 1))
    ssel = din("ssel", (128, 2 * 4 * 128), BF16)
    # ---- outputs ----
    res_d = nc.dram_tensor("res", (128, L), F32, kind="ExternalOutput")
    emb_d = nc.dram_tensor("embdbg", (E, 2 * L), F32, kind="ExternalOutput")

    with tile.TileContext(nc) as tc, contextlib.ExitStack() as ctx:
        const = ctx.enter_context(tc.tile_pool(name="const", bufs=1))
        sc = ctx.enter_context(tc.tile_pool(name="sc", bufs=2))
        scs = ctx.enter_context(tc.tile_pool(name="scs", bufs=4))
        hpool = ctx.enter_context(tc.tile_pool(name="hp", bufs=3))
        vap = ctx.enter_context(tc.tile_pool(name="vap", bufs=2))
        wpool = ctx.enter_context(tc.tile_pool(name="wp", bufs=2))
        ps = ctx.enter_context(tc.tile_pool(name="ps", bufs=2, space="PSUM"))
        ps2 = ctx.enter_context(tc.tile_pool(name="ps2", bufs=2, space="PSUM"))
        psb = ctx.enter_context(tc.tile_pool(name="psb", bufs=1, space="PSUM"))

        def ld(dram, p, f, d=F32, pool=const):
            nm = "ld_" + dram.name
            t = pool.tile([p, f], d, tag=nm, name=nm)
            nc.sync.dma_start(t[:], dram.ap())
            return t

        # ---------- load weights ----------
        seqT_t = ld(seqT, 4, 2 * L, BF16)
        w1Tt_t = ld(w1Tt, 4, 9 * D, BF16)
        w2Tt_t = ld(w2Tt, D, 9 * D, BF16)
        bn1g_t = ld(bn1g, D, 1); bn1b_t = ld(bn1b, D, 1)
        bn2g_t = ld(bn2g, D, 1); bn2b_t = ld(bn2b, D, 1)
        rvf_t = ld(rvec_f, 1, L)
        iwTq_t = ld(iwTq, E, 3 * E, BF16)
        iwTk_t = ld(iwTk, E, 3 * E, BF16)
        iwTv_t = ld(iwTv, E, 3 * E, BF16)
        qkb_t = ld(qkb, E, 6)
        owT_t = ld(owTd, 64, 3 * 2 * E, BF16)
        obe_t = ld(obe, E, 3)
        ln1g_t = ld(ln1g, E, 3); ln1b_t = ld(ln1b, E, 3)
        ln2g_t = ld(ln2g, E, 3); ln2b_t = ld(ln2b, E, 3)
        w1Tf_t = ld(w1Tf, E, 3 * FF, BF16)
        fb1_t = ld(fb1, E, 3 * 16)
        w2Tf_t = ld(w2Tf, E, 3 * 16 * E, BF16)
        fb2_t = ld(fb2, E, 3)
        ssel_t = ld(ssel, 128, 2 * 4 * 128, BF16)
        rcol_t = ld(rcol, 128, 1)

        # constants
        AVG = const.tile([128, 128], BF16); nc.vector.memset(AVG[:], 1.0 / 128)
        ONE1 = const.tile([1, 128], BF16); nc.vector.memset(ONE1[:], 1.0)
        ONE1F = const.tile([1, 128], F32); nc.vector.memset(ONE1F[:], 1.0)
        ONES64 = const.tile([128, 64], BF16); nc.vector.memset(ONES64[:], 1.0)
        ONESW = const.tile([128, 256], BF16)
        nc.vector.memset(ONESW[:], 0.0)
        nc.vector.memset(ONESW[:, 128:129], 1.0)
        IDN = const.tile([128, 128], BF16)
        make_identity(nc, IDN[:])
        EPSC = const.tile([128, 1], F32); nc.vector.memset(EPSC[:], EPS)
        ZERC = const.tile([128, 1], F32); nc.vector.memset(ZERC[:], 0.0)
        PAIR = [const.tile([128, 258], BF16, tag=f"pair{i}", name=f"pair{i}") for i in range(2)]
        for p in PAIR:
            nc.vector.memset(p[:], 0.0)

        # pos rows duplicated into even/odd partitions (p = 2r + b)
        w256 = const.tile([128, L], F32)
        w257 = const.tile([128, L], F32)
        for tdup, dsrc in ((w256, lcp256), (w257, lcp257)):
            v = tdup[:].rearrange("(r two) j -> r two j", two=2)
            nc.sync.dma_start(v[:, 0, :], dsrc.ap())
            nc.sync.dma_start(v[:, 1, :], dsrc.ap())
        prb = ps.tile([128, L], F32, tag="psm0")
        nc.tensor.matmul(prb[:], ONE1F[:], rvf_t[:], start=True, stop=True)
        t257 = const.tile([128, L], F32)
        nc.vector.tensor_mul(t257[:], w257[:], prb[:])
        posdup = const.tile([128, L], F32)
        nc.vector.scalar_tensor_tensor(
            out=posdup[:], in0=w256[:], scalar=rcol_t[:], in1=t257[:],
            op0=AL.mult, op1=AL.add)

        # ---------- conv front-end ----------
        def bn_relu_block(y, g_t, b_t, outs):
            # y: list of 2 sbuf (D, L) f32; outs: list of (tile, dtype) pairs
            stats = []
            for b in range(2):
                s_b = scs.tile([D, 1], F32, tag="bnst")
                nc.vector.tensor_reduce(s_b[:], y[b][:], mybir.AxisListType.X, AL.add)
                ysq = sc.tile([D, L], F32, tag="ysq")
                q_b = scs.tile([D, 1], F32, tag="bnst2")
                nc.vector.scalar_tensor_tensor(
                    out=ysq[:], in0=y[b][:], scalar=0.0, in1=y[b][:],
                    op0=AL.bypass, op1=AL.mult, accum_out=q_b[:])
                stats.append((s_b, q_b))
            ssum = scs.tile([D, 1], F32)
            nc.vector.tensor_add(ssum[:], stats[0][0][:], stats[1][0][:])
            qsum = scs.tile([D, 1], F32)
            nc.vector.tensor_add(qsum[:], stats[0][1][:], stats[1][1][:])
            mean = scs.tile([D, 1], F32)
            nc.vector.tensor_scalar(out=mean[:], in0=ssum[:], scalar1=1.0 / (2 * L),
                                    scalar2=None, op0=AL.mult)
            ex2 = scs.tile([D, 1], F32)
            nc.vector.tensor_scalar(out=ex2[:], in0=qsum[:], scalar1=1.0 / (2 * L),
                                    scalar2=None, op0=AL.mult)
            m2 = scs.tile([D, 1], F32)
            nc.vector.tensor_mul(m2[:], mean[:], mean[:])
            var = scs.tile([D, 1], F32)
            nc.vector.tensor_sub(var[:], ex2[:], m2[:])
            lg = scs.tile([D, 1], F32)
            nc.scalar.activation(lg[:], var[:], AF.Ln, bias=EPSC[0:D, :])
            isd = scs.tile([D, 1], F32)
            nc.scalar.activation(isd[:], lg[:], AF.Exp, scale=-0.5)
            alpha = scs.tile([D, 1], F32)
            nc.vector.tensor_mul(alpha[:], g_t[:], isd[:])
            tmid = scs.tile([D, 1], F32)
            nc.vector.tensor_mul(tmid[:], mean[:], alpha[:])
            beta = scs.tile([D, 1], F32)
            nc.vector.tensor_sub(beta[:], b_t[:], tmid[:])
            for b in range(2):
                for o in outs[b]:
                    nc.scalar.activation(o, y[b][:], AF.Relu,
                                         bias=beta[:], scale=alpha[:])

        # conv1
        y1 = []
        for b in range(2):
            cp = ps.tile(tag="psm0", shape=[D, L], dtype=F32)
            for t in range(9):
                o = 2 * t - 8
                lo, hi = max(0, -o), L - max(0, o)
                nc.tensor.matmul(
                    cp[:, lo:hi], w1Tt_t[:, t * D:(t + 1) * D],
                    seqT_t[:, b * L + lo + o: b * L + hi + o],
                    start=(t == 0), stop=(t == 8), skip_group_check=True)
            yb = sc.tile([D, L], F32, tag=f"y1_{b}")
            nc.scalar.copy(yb[:], cp[:])
            y1.append(yb)
        x1bf = [sc.tile([D, L], BF16, tag=f"x1bf{i}", name=f"x1bf{i}") for i in range(2)]
        bn_relu_block(y1, bn1g_t, bn1b_t, [[x1bf[b][:]] for b in range(2)])

        # conv2 (+ pos1d row -> (128, L) tiles)
        xcur_bf = [vap.tile([E, L], BF16, tag=f"xbf{i}", name=f"xbf{i}") for i in range(2)]
        xcur_f = [vap.tile([E, L], F32, tag=f"xf{i}", name=f"xf{i}") for i in range(2)]
        y2 = []
        for b in range(2):
            cp = ps.tile(tag="psm0", shape=[D, L], dtype=F32)
            for t in range(9):
                o = 2 * t - 8
                lo, hi = max(0, -o), L - max(0, o)
                nc.tensor.matmul(
                    cp[:, lo:hi], w2Tt_t[:, t * D:(t + 1) * D],
                    x1bf[b][:, lo + o: hi + o],
                    start=(t == 0), stop=(t == 8), skip_group_check=True)
            yb = sc.tile([D, L], F32, tag=f"y2_{b}")
            nc.scalar.copy(yb[:], cp[:])
            y2.append(yb)
        bn_relu_block(y2, bn2g_t, bn2b_t,
                      [[xcur_bf[b][0:D, :], xcur_f[b][0:D, :]] for b in range(2)])
        for b in range(2):
            nc.sync.dma_start(xcur_bf[b][D:E, :], rvec_bf.ap())
            nc.sync.dma_start(xcur_f[b][D:E, :], rvec_f.ap())

        # ---------- transformer layers ----------
        def layer_norm(x_f, g_col, b_col, out_bf, out_f, ptag="psm0"):
            xb = sc.tile([E, L], BF16, tag="lnxb")
            nc.vector.tensor_copy(xb[:], x_f[:])
            xsq = sc.tile([E, L], BF16, tag="lnxsq")
            nc.vector.scalar_tensor_tensor(
                out=xsq[:], in0=x_f[:], scalar=0.0, in1=x_f[:],
                op0=AL.bypass, op1=AL.mult)
            pm = ps.tile([E, L], F32, tag=ptag)
            nc.tensor.matmul(pm[:], AVG[:], xb[:], start=True, stop=True)
            pq = ps.tile([E, L], F32, tag=ptag)
            nc.tensor.matmul(pq[:], AVG[:], xsq[:], start=True, stop=True)
            v1 = sc.tile([E, L], F32, tag="lnv1")
            nc.vector.tensor_sub(v1[:], x_f[:], pm[:])
            m_sb = scs.tile([1, L], F32, tag="lnrow")
            nc.vector.tensor_copy(m_sb[:], pm[0:1, :])
            m2 = scs.tile([1, L], F32, tag="lnrow2")
            nc.vector.tensor_mul(m2[:], m_sb[:], m_sb[:])
            var_r = scs.tile([1, L], F32, tag="lnrow3")
            nc.vector.tensor_sub(var_r[:], pq[0:1, :], m2[:])
            lg = scs.tile([1, L], F32, tag="lnrow4")
            nc.scalar.activation(lg[:], var_r[:], AF.Ln, bias=EPSC[0:1, :])
            rstd = scs.tile([1, L], F32, tag="lnrow5")
            nc.scalar.activation(rstd[:], lg[:], AF.Exp, scale=-0.5)
            rstd_bf = scs.tile([1, L], BF16, tag="lnrow6")
            nc.vector.tensor_copy(rstd_bf[:], rstd[:])
            pb = ps.tile([E, L], F32, tag=ptag)
            nc.tensor.matmul(pb[:], ONE1[:], rstd_bf[:], start=True, stop=True)
            v2 = sc.tile([E, L], F32, tag="lnv2")
            nc.vector.tensor_mul(v2[:], v1[:], pb[:])
            nc.vector.scalar_tensor_tensor(
                out=out_bf, in0=v2[:], scalar=g_col, in1=b_col.broadcast_to((E, L)),
                op0=AL.mult, op1=AL.add)
            nc.vector.scalar_tensor_tensor(
                out=out_f, in0=v2[:], scalar=g_col, in1=b_col.broadcast_to((E, L)),
                op0=AL.mult, op1=AL.add)

        for l in range(3):
            for b in range(2):
                x_bf, x_f = xcur_bf[b], xcur_f[b]
                # q, k
                pq = ps.tile([E, L], F32, tag="psm0")
                nc.tensor.matmul(pq[:], iwTq_t[:, l * E:(l + 1) * E], x_bf[:],
                                 start=True, stop=True)
                q_s = sc.tile([E, L], BF16, tag="qs")
                nc.vector.tensor_scalar(out=q_s[:], in0=pq[:],
                                        scalar1=qkb_t[:, 2 * l:2 * l + 1],
                                        scalar2=None, op0=AL.add)
                pk = ps.tile([E, L], F32, tag="psm0")
                nc.tensor.matmul(pk[:], iwTk_t[:, l * E:(l + 1) * E], x_bf[:],
                                 start=True, stop=True)
                k_s = sc.tile([E, L], BF16, tag="ks")
                nc.vector.tensor_scalar(out=k_s[:], in0=pk[:],
                                        scalar1=qkb_t[:, 2 * l + 1:2 * l + 2],
                                        scalar2=None, op0=AL.add)
                # vT (m on partitions): chunk c cols [128c, 128c+128) = all d
                vT_s = sc.tile([128, 4 * 128], BF16, tag="vts")
                for c in range(4):
                    pv = ps2.tile([128, E], F32, tag="pa", bufs=2)
                    nc.tensor.matmul(pv[:], x_bf[:, c * 128:(c + 1) * 128],
                                     iwTv_t[:, l * E:(l + 1) * E],
                                     start=True, stop=True)
                    nc.scalar.copy(vT_s[:, c * 128:(c + 1) * 128], pv[:])
                # attention per head
                pproj = ps.tile([E, L], F32, tag="psm0")
                for h in range(2):
                    po = ps2.tile([64, L], F32, tag="po", bufs=2)
                    psum_s = ps2.tile([64, L], F32, tag="psums", bufs=1)
                    for c in range(4):
                        pa = ps2.tile([128, L], F32, tag="pa", bufs=2)
                        nc.tensor.matmul(
                            pa[:], k_s[64 * h:64 * h + 64, c * 128:(c + 1) * 128],
                            q_s[64 * h:64 * h + 64, :], start=True, stop=True)
                        at = sc.tile([128, L], BF16, tag="at")
                        nc.scalar.activation(at[:], pa[:], AF.Exp, scale=0.125)
                        nc.tensor.matmul(
                            po[:], vT_s[:, c * 128 + 64 * h:c * 128 + 64 * h + 64],
                            at[:], start=(c == 0), stop=(c == 3),
                            skip_group_check=True)
                        nc.tensor.matmul(
                            psum_s[:], ONES64[:], at[:],
                            start=(c == 0), stop=(c == 3), skip_group_check=True)
                    lgs = sc.tile([64, L], F32, tag="lgs")
                    nc.scalar.activation(lgs[:], psum_s[:], AF.Ln, bias=ZERC[0:64, :])
                    inv_bf = sc.tile([64, L], BF16, tag="invbf")
                    nc.scalar.activation(inv_bf[:], lgs[:], AF.Exp, scale=-1.0)
                    o_s = sc.tile([64, L], BF16, tag="os")
                    nc.scalar.copy(o_s[:], po[:])
                    on_s = sc.tile([64, L], BF16, tag="ons")
                    nc.vector.tensor_mul(on_s[:], o_s[:], inv_bf[:])
                    nc.tensor.matmul(
                        pproj[:],
                        owT_t[:, (l * 2 + h) * E:(l * 2 + h + 1) * E],
                        on_s[:], start=(h == 0), stop=(h == 1),
                        skip_group_check=True)
                x1_f = vap.tile([E, L], F32, tag="x1f")
                nc.vector.scalar_tensor_tensor(
                    out=x1_f[:], in0=pproj[:], scalar=obe_t[:, l:l + 1],
                    in1=x_f[:], op0=AL.add, op1=AL.add)
                x1ln_bf = vap.tile([E, L], BF16, tag="x1lnbf")
                x1ln_f = vap.tile([E, L], F32, tag="x1lnf")
                layer_norm(x1_f, ln1g_t[:, l:l + 1], ln1b_t[:, l:l + 1],
                           x1ln_bf[:], x1ln_f[:], ptag="psm0")
                # FFN
                pf2 = ps.tile([E, L], F32, tag="psm0")
                for c in range(16):
                    pf1 = ps2.tile([E, L], F32, tag="pa", bufs=2)
                    nc.tensor.matmul(
                        pf1[:], w1Tf_t[:, l * FF + c * E: l * FF + (c + 1) * E],
                        x1ln_bf[:], start=True, stop=True)
                    h_bf = hpool.tile([E, L], BF16, tag="hbf")
                    nc.scalar.activation(h_bf[:], pf1[:], AF.Relu,
                                         bias=fb1_t[:, l * 16 + c: l * 16 + c + 1])
                    nc.tensor.matmul(
                        pf2[:], w2Tf_t[:, (l * 16 + c) * E:(l * 16 + c + 1) * E],
                        h_bf[:], start=(c == 0), stop=(c == 15),
                        skip_group_check=True)
                x2_f = vap.tile([E, L], F32, tag="x2f")
                nc.vector.scalar_tensor_tensor(
                    out=x2_f[:], in0=pf2[:], scalar=fb2_t[:, l:l + 1],
                    in1=x1ln_f[:], op0=AL.add, op1=AL.add)
                nxt_bf = vap.tile([E, L], BF16, tag=f"xbf{b}")
                nxt_f = vap.tile([E, L], F32, tag=f"xf{b}")
                layer_norm(x2_f, ln2g_t[:, l:l + 1], ln2b_t[:, l:l + 1],
                           nxt_bf[:], nxt_f[:], ptag="psm0")
                xcur_bf[b], xcur_f[b] = nxt_bf, nxt_f

        emb_bf, emb_f = xcur_bf, xcur_f
        for b in range(2):
            nc.sync.dma_start(emb_d.ap()[:, b * L:(b + 1) * L], emb_f[b][:])

        # ---------- emb_il: emb columns for this core's rows, interleaved ----
        # emb_il[:, 2r+b] = emb_bf[b][:, r0+r], via transpose + selection matmul
        pil = ps.tile([128, 128], F32, tag="psm0")
        first = True
        for b in range(2):
            for c in range(4):
                pt = ps2.tile([128, 128], BF16, tag="pa", bufs=2)
                nc.tensor.transpose(pt[:], emb_bf[b][:, c * 128:(c + 1) * 128],
                                    IDN[:])
                et = sc.tile([128, 128], BF16, tag="et")
                nc.scalar.copy(et[:], pt[:])
                nc.tensor.matmul(
                    pil[:], et[:],
                    ssel_t[:, (b * 4 + c) * 128:(b * 4 + c + 1) * 128],
                    start=first, stop=(b == 1 and c == 3), skip_group_check=True)
                first = False
        emb_il = const.tile([128, 128], BF16)
        nc.scalar.copy(emb_il[:], pil[:])

        # ---------- LC main loop ----------
        acc = psb.tile([128, L], F32)
        nchunks = RPC // WCHUNK
        first_mm = True
        for ch in range(nchunks):
            wa_t = wpool.tile([128, WCHUNK * L], BF16, tag="wa")
            nc.sync.dma_start(
                wa_t[:], wa_d.ap()[:, ch * WCHUNK * L:(ch + 1) * WCHUNK * L])
            wb_t = wpool.tile([128, WCHUNK * L], BF16, tag="wb")
            nc.sync.dma_start(
                wb_t[:], wb_d.ap()[:, ch * WCHUNK * L:(ch + 1) * WCHUNK * L])
            for rr in range(WCHUNK):
                r = ch * WCHUNK + rr
                wa_row = wa_t[:, rr * L:(rr + 1) * L]
                wb_row = wb_t[:, rr * L:(rr + 1) * L]
                # prod_a per batch: emb (.) Wa_row, reduce over c via window mm
                for b in range(2):
                    va = scs.tile([128, L], BF16, tag="va")
                    nc.vector.tensor_mul(va[:], emb_bf[b][:], wa_row)
                    p = 2 * r + b
                    nc.tensor.matmul(acc[:], ONESW[:, 128 - p:256 - p], va[:],
                                     start=first_mm, stop=False,
                                     skip_group_check=True)
                    first_mm = False
                # prod_b both batches at once: emb cols at window pos 2r, 2r+1
                pair = PAIR[r % 2]
                nc.vector.tensor_copy(pair[:, 128:130], emb_il[:, 2 * r:2 * r + 2])
                nc.tensor.matmul(acc[:], pair[:, 128 - 2 * r:256 - 2 * r], wb_row,
                                 start=False, stop=(r == RPC - 1),
                                 skip_group_check=True)
        res_t = const.tile([128, L], F32)
        nc.vector.tensor_add(res_t[:], acc[:], posdup[:])
        nc.sync.dma_start(res_d.ap(), res_t[:])

    nc.compile()
    return nc


def _prep_inputs(inputs):
    f32 = np.float32

    def bf(x):
        return np.ascontiguousarray(x.astype(BF))

    def f(x):
        return np.ascontiguousarray(x.astype(f32))

    seq = inputs["seq"]  # (B, L, 4)
    com = {}
    com["seqT"] = bf(np.concatenate([seq[b].T for b in range(B)], axis=1))
    com["w1Tt"] = bf(np.concatenate(
        [inputs["conv1_w"][:, :, t].T for t in range(9)], axis=1))
    com["w2Tt"] = bf(np.concatenate(
        [inputs["conv2_w"][:, :, t].T for t in range(9)], axis=1))
    for nm in ("bn1_g", "bn1_b", "bn2_g", "bn2_b"):
        com[nm.replace("_", "")] = f(inputs[nm][:, None])
    rv = (np.arange(1, L + 1, dtype=f32) / L)[None, :]
    com["rvec_bf"] = bf(rv); com["rvec_f"] = f(rv)
    iw = inputs["attn_in_w"]   # (3, 3E, E)
    ib = inputs["attn_in_b"]   # (3, 3E)
    com["iwTq"] = bf(np.concatenate([iw[l][0:E].T for l in range(3)], axis=1))
    com["iwTk"] = bf(np.concatenate([iw[l][E:2 * E].T for l in range(3)], axis=1))
    com["iwTv"] = bf(np.concatenate([iw[l][2 * E:3 * E].T for l in range(3)],
                                    axis=1))
    qkb = np.zeros((E, 6), f32)
    for l in range(3):
        qkb[:, 2 * l] = ib[l, 0:E]
        qkb[:, 2 * l + 1] = ib[l, E:2 * E]
    com["qkb"] = qkb
    ow = inputs["attn_out_w"]; ob = inputs["attn_out_b"]
    com["owT"] = bf(np.concatenate([ow[l][:, 64 * h:64 * h + 64].T for l in range(3) for h in range(2)], axis=1))
    obe = np.stack([ob[l] + ow[l] @ ib[l, 2 * E:3 * E] for l in range(3)], axis=1)
    com["obe"] = f(obe)
    for nm, key in (("ln1g", "ln1_g"), ("ln1b", "ln1_b"),
                    ("ln2g", "ln2_g"), ("ln2b", "ln2_b")):
        com[nm] = f(inputs[key].T)  # (E, 3)
    w1 = inputs["ffn_w1"]; w2 = inputs["ffn_w2"]
    com["w1Tf"] = bf(np.concatenate([w1[l].T for l in range(3)], axis=1))
    com["fb1"] = f(np.concatenate(
        [inputs["ffn_b1"][l].reshape(16, E).T for l in range(3)], axis=1))
    com["w2Tf"] = bf(np.concatenate(
        [w2[l][:, c * E:(c + 1) * E].T for l in range(3) for c in range(16)],
        axis=1))
    com["fb2"] = f(inputs["ffn_b2"].T)

    lc = np.asarray(inputs["lc_w"], dtype=f32)  # (258, L, L)
    idx = np.arange(L)
    mask = (idx[None, :] <= idx[:, None])[None]  # (1, L, L): j <= i
    Wa = np.where(mask, lc[0:E], lc[E:2 * E]).astype(BF)
    Wb = np.where(mask, lc[E:2 * E], lc[0:E]).astype(BF)

    in_maps = []
    for k in range(NCORES):
        r0 = k * RPC
        m = dict(com)
        m["wa"] = np.ascontiguousarray(
            Wa[:, r0:r0 + RPC, :]).reshape(128, RPC * L)
        m["wb"] = np.ascontiguousarray(
            Wb[:, r0:r0 + RPC, :]).reshape(128, RPC * L)
        m["lcp256"] = f(lc[2 * E, r0:r0 + RPC, :])
        m["lcp257"] = f(lc[2 * E + 1, r0:r0 + RPC, :])
        m["rcol"] = f(np.repeat((r0 + np.arange(RPC) + 1) / L, 2)[:, None])
        ss = np.zeros((128, 8 * 128), f32)
        mm = np.arange(128)
        for b in range(2):
            sel = mm[mm % 2 == b]
            Sb = np.zeros((L, 128), f32)
            Sb[r0 + sel // 2, sel] = 1.0
            for c in range(4):
                ss[:, (b * 4 + c) * 128:(b * 4 + c + 1) * 128] = \
                    Sb[c * 128:(c + 1) * 128, :]
        m["ssel"] = bf(ss)
        in_maps.append(m)
    return in_maps


def kernel(**inputs):
    if "nc" not in _cached:
        _cached["nc"] = _build()
    nc = _cached["nc"]
    in_maps = _prep_inputs({k: np.asarray(v) for k, v in inputs.items()})
    out = run_bass_kernel_spmd(nc, in_maps, list(range(NCORES)))
    _cached["last"] = out
    contact = np.zeros((B, L, L), np.float32)
    for k in range(NCORES):
        r0 = k * RPC
        R = out.results[k]["res"].reshape(RPC, 2, L)
        contact[0, r0:r0 + RPC] = R[:, 0]
        contact[1, r0:r0 + RPC] = R[:, 1]
    return ((contact + contact.transpose(0, 2, 1)) * 0.5).astype(np.float32)

